# revision 5
# baseline (speedup 1.0000x reference)
"""Weighted Pearson correlation (six fused global reductions) on 8 trn2 cores.

Sharding: data-parallel over the flat N=2^25 dimension; each core reduces its
4M-element shard to a small set of partial sums which the host combines.

Per-core engine split (targets the ~140us/core HBM roofline for the 48MiB shard):
  - DVE    : 3 fused tensor_tensor_reduce ops/tile -> products nx=n*x, ny=n*y
             plus the cancellation-sensitive sums (sum_nx, sum_ny, sum_nxy),
             all in fp32.
  - GPSIMD : 1 tensor_tensor mul/tile -> nxx = nx*x.
  - ACT    : 2 activation-Copy ops/tile with accum_out -> sum_n, sum_nxx
             (free-axis fp32 accumulation; ACT is otherwise idle).
  - PE     : fp32 "diagonal" matmuls: diag(ny_chunk^T @ y_chunk) accumulated in
             one PSUM bank across all chunks/tiles; the diagonal of the final
             128x128 block is the per-column partial of sum_ny2.
Host: gathers per-core partials (a few KB), reduces in float64, applies the
12-flop correlation formula.
"""

import numpy as np

import concourse.bass as bass
import concourse.bacc as bacc
import concourse.tile as tile
from concourse import mybir
from concourse.bass_utils import run_bass_kernel_spmd

N_TOTAL = 33554432  # 2^25
N_CORES = 8
P = 128  # SBUF partitions

# Per-core shard: 4194304 elements = T tiles of [P, F]
F = 2048
T = N_TOTAL // N_CORES // (P * F)  # 16

_F32 = mybir.dt.float32
_MULT = mybir.AluOpType.mult
_ADD = mybir.AluOpType.add
_COPY = mybir.ActivationFunctionType.Copy


def build_nc(tiles=T, free=F, in_bufs=3, prod_bufs=2):
    """Build the per-core Bass program. All 8 cores run this same program on
    their own shard (inputs shaped [tiles, 128, free])."""
    f = free
    c128 = f // 128  # stationary operand is at most 128 columns

    nc = bacc.Bacc(None)
    xs = nc.dram_tensor("xs", [tiles, P, f], _F32, kind="ExternalInput")
    ys = nc.dram_tensor("ys", [tiles, P, f], _F32, kind="ExternalInput")
    ns = nc.dram_tensor("ns", [tiles, P, f], _F32, kind="ExternalInput")
    # Partial-sum outputs: host finishes the reduction.
    # rows: 0=sum_nx, 1=sum_ny, 2=sum_nxy, 3=sum_n, 4=sum_nxx
    o_stats = nc.dram_tensor("o_stats", [5, P, tiles], _F32, kind="ExternalOutput")
    o_diag = nc.dram_tensor("o_diag", [P, P], _F32, kind="ExternalOutput")

    with tile.TileContext(nc) as tc:
        with (
            tc.tile_pool(name="ins", bufs=in_bufs) as inp,
            tc.tile_pool(name="prods", bufs=prod_bufs) as prods,
            tc.tile_pool(name="acc", bufs=1) as accp,
            tc.tile_pool(name="psum", bufs=1, space="PSUM") as psump,
        ):
            stats_x = accp.tile([P, tiles], _F32, tag="sx")
            stats_y = accp.tile([P, tiles], _F32, tag="sy")
            stats_xy = accp.tile([P, tiles], _F32, tag="sxy")
            stats_n = accp.tile([P, tiles], _F32, tag="sn")
            stats_xx = accp.tile([P, tiles], _F32, tag="sxx")

            psum_yy = psump.tile([P, P], _F32, tag="pyy")

            for t in range(tiles):
                x_t = inp.tile([P, f], _F32, tag="x")
                y_t = inp.tile([P, f], _F32, tag="y")
                n_t = inp.tile([P, f], _F32, tag="n")
                nc.sync.dma_start(out=x_t[:], in_=xs[t])
                nc.sync.dma_start(out=y_t[:], in_=ys[t])
                nc.sync.dma_start(out=n_t[:], in_=ns[t])

                nx_t = prods.tile([P, f], _F32, tag="nx")
                ny_t = prods.tile([P, f], _F32, tag="ny")
                junk_t = prods.tile([P, f], _F32, tag="junk")
                nxx_t = prods.tile([P, f], _F32, tag="nxx")
                ajunk_t = prods.tile([P, f], _F32, tag="ajunk")
                ajunk2_t = prods.tile([P, f], _F32, tag="ajunk2")

                # DVE: products + fused free-axis sums (custom-DVE op:
                # out = (in0*1+0)*in1, accum_out = sum(out)).
                nc.vector.affine_mul_reduce(
                    out=nx_t[:], accum_out=stats_x[:, t : t + 1],
                    in0=x_t[:], in1=n_t[:], scale=1.0, bias=0.0,
                )
                nc.vector.affine_mul_reduce(
                    out=ny_t[:], accum_out=stats_y[:, t : t + 1],
                    in0=y_t[:], in1=n_t[:], scale=1.0, bias=0.0,
                )
                nc.vector.affine_mul_reduce(
                    out=junk_t[:], accum_out=stats_xy[:, t : t + 1],
                    in0=nx_t[:], in1=y_t[:], scale=1.0, bias=0.0,
                )

                # GPSIMD: nxx = nx * x.
                nc.gpsimd.tensor_tensor(
                    out=nxx_t[:], in0=nx_t[:], in1=x_t[:], op=_MULT
                )

                # ACT: free-axis sums of n and nxx via Copy + accumulate.
                nc.scalar.activation(
                    out=ajunk_t[:], in_=n_t[:], func=_COPY,
                    accum_out=stats_n[:, t : t + 1],
                )
                nc.scalar.activation(
                    out=ajunk2_t[:], in_=nxx_t[:], func=_COPY,
                    accum_out=stats_xx[:, t : t + 1],
                )

                # PE: fp32 diag-matmuls -> sum over chunks of ny_chunk^T @ y_chunk;
                # only the accumulated diagonal is meaningful (= sum n*y^2).
                for c in range(c128):
                    s = bass.ts(c, 128)
                    nc.tensor.matmul(
                        psum_yy[:],
                        ny_t[:, s],
                        y_t[:, s],
                        start=(t == 0 and c == 0),
                        stop=(t == tiles - 1 and c == c128 - 1),
                    )

            nc.sync.dma_start(out=o_stats[0], in_=stats_x[:])
            nc.sync.dma_start(out=o_stats[1], in_=stats_y[:])
            nc.sync.dma_start(out=o_stats[2], in_=stats_xy[:])
            nc.sync.dma_start(out=o_stats[3], in_=stats_n[:])
            nc.sync.dma_start(out=o_stats[4], in_=stats_xx[:])
            # DMA cannot read PSUM: bounce through SBUF.
            sb_yy = accp.tile([P, P], _F32, tag="sbyy")
            nc.vector.tensor_copy(out=sb_yy[:], in_=psum_yy[:])
            nc.sync.dma_start(out=o_diag[:], in_=sb_yy[:])

    nc.finalize()
    return nc


_NC_CACHE = None


def _get_nc():
    global _NC_CACHE
    if _NC_CACHE is None:
        _NC_CACHE = build_nc()
    return _NC_CACHE


def combine_partials(results):
    """Host-side all-reduce of the per-core partials + correlation formula."""
    sum_x = sum_y = sum_prod = sum_n = sum_x2 = sum_y2 = 0.0
    for r in results:
        st = np.asarray(r["o_stats"], dtype=np.float64)
        sum_x += st[0].sum()
        sum_y += st[1].sum()
        sum_prod += st[2].sum()
        sum_n += st[3].sum()
        sum_x2 += st[4].sum()
        sum_y2 += np.diag(np.asarray(r["o_diag"], dtype=np.float64)).sum()
    numerator = sum_n * sum_prod - sum_x * sum_y
    denominator = np.sqrt(sum_n * sum_x2 - sum_x * sum_x) * np.sqrt(
        sum_n * sum_y2 - sum_y * sum_y
    )
    return np.asarray([numerator / denominator], dtype=np.float32)


def kernel(xs, ys, ns, **run_kwargs):
    xs = np.ascontiguousarray(np.asarray(xs, dtype=np.float32)).reshape(
        N_CORES, T, P, F
    )
    ys = np.ascontiguousarray(np.asarray(ys, dtype=np.float32)).reshape(
        N_CORES, T, P, F
    )
    ns = np.ascontiguousarray(np.asarray(ns, dtype=np.float32)).reshape(
        N_CORES, T, P, F
    )
    in_maps = [
        {"xs": xs[c], "ys": ys[c], "ns": ns[c]} for c in range(N_CORES)
    ]
    res = run_bass_kernel_spmd(
        _get_nc(), in_maps, core_ids=list(range(N_CORES)), **run_kwargs
    )
    return combine_partials(res.results)


# revision 7
# speedup vs baseline: 3.2185x; 3.2185x over previous
"""Weighted Pearson correlation (six fused global reductions) on 8 trn2 cores.

Sharding: data-parallel over the flat N=2^25 dimension; each core reduces its
4M-element shard to a small set of partial sums which the host combines.

Per-core engine split (targets the ~140us/core HBM roofline for the 48MiB shard):
  - DVE    : 3 fused tensor_tensor_reduce ops/tile -> products nx=n*x, ny=n*y
             plus the cancellation-sensitive sums (sum_nx, sum_ny, sum_nxy),
             all in fp32.
  - GPSIMD : 1 tensor_tensor mul/tile -> nxx = nx*x.
  - ACT    : 2 activation-Copy ops/tile with accum_out -> sum_n, sum_nxx
             (free-axis fp32 accumulation; ACT is otherwise idle).
  - PE     : fp32 "diagonal" matmuls: diag(ny_chunk^T @ y_chunk) accumulated in
             one PSUM bank across all chunks/tiles; the diagonal of the final
             128x128 block is the per-column partial of sum_ny2.
Host: gathers per-core partials (a few KB), reduces in float64, applies the
12-flop correlation formula.
"""

import numpy as np

import concourse.bass as bass
import concourse.bacc as bacc
import concourse.tile as tile
from concourse import mybir
from concourse.bass_utils import run_bass_kernel_spmd

N_TOTAL = 33554432  # 2^25
N_CORES = 8
P = 128  # SBUF partitions

# Per-core shard: 4194304 elements = T tiles of [P, F]
F = 2048
T = N_TOTAL // N_CORES // (P * F)  # 16

_F32 = mybir.dt.float32
_MULT = mybir.AluOpType.mult
_ADD = mybir.AluOpType.add
_COPY = mybir.ActivationFunctionType.Copy


def build_nc(tiles=T, free=F, in_bufs=3, prod_bufs=2, rounds=1):
    """Build the per-core Bass program. All 8 cores run this same program on
    their own shard (inputs shaped [tiles, 128, free])."""
    f = free
    c128 = f // 128  # stationary operand is at most 128 columns

    nc = bacc.Bacc(None)
    xs = nc.dram_tensor("xs", [tiles, P, f], _F32, kind="ExternalInput")
    ys = nc.dram_tensor("ys", [tiles, P, f], _F32, kind="ExternalInput")
    ns = nc.dram_tensor("ns", [tiles, P, f], _F32, kind="ExternalInput")
    # Partial-sum outputs: host finishes the reduction.
    # rows: 0=sum_nx, 1=sum_ny, 2=sum_nxy, 3=sum_n, 4=sum_nxx
    o_stats = nc.dram_tensor("o_stats", [5, P, tiles], _F32, kind="ExternalOutput")
    o_diag = nc.dram_tensor("o_diag", [P, P], _F32, kind="ExternalOutput")
    # Tiny passthrough (tick->tock) so a bench harness can chain executions
    # with a data dependency; costs two 4KB DMAs.
    tick = nc.dram_tensor("tick", [P, 8], _F32, kind="ExternalInput")
    tock = nc.dram_tensor("tock", [P, 8], _F32, kind="ExternalOutput")

    with tile.TileContext(nc) as tc:
        with (
            tc.tile_pool(name="ins", bufs=in_bufs) as inp,
            tc.tile_pool(name="prods", bufs=prod_bufs) as prods,
            tc.tile_pool(name="acc", bufs=1) as accp,
            tc.tile_pool(name="psum", bufs=1, space="PSUM") as psump,
        ):
            stats_x = accp.tile([P, tiles], _F32, tag="sx")
            stats_y = accp.tile([P, tiles], _F32, tag="sy")
            stats_xy = accp.tile([P, tiles], _F32, tag="sxy")
            stats_n = accp.tile([P, tiles], _F32, tag="sn")
            stats_xx = accp.tile([P, tiles], _F32, tag="sxx")

            psum_yy = psump.tile([P, P], _F32, tag="pyy")

            n_iter = rounds * tiles
            for rt in range(n_iter):
                t = rt % tiles
                x_t = inp.tile([P, f], _F32, tag="x")
                y_t = inp.tile([P, f], _F32, tag="y")
                n_t = inp.tile([P, f], _F32, tag="n")
                nc.sync.dma_start(out=x_t[:], in_=xs[t])
                nc.sync.dma_start(out=y_t[:], in_=ys[t])
                nc.sync.dma_start(out=n_t[:], in_=ns[t])

                nx_t = prods.tile([P, f], _F32, tag="nx")
                ny_t = prods.tile([P, f], _F32, tag="ny")
                junk_t = prods.tile([P, f], _F32, tag="junk")
                nxx_t = prods.tile([P, f], _F32, tag="nxx")
                ajunk_t = prods.tile([P, f], _F32, tag="ajunk")
                ajunk2_t = prods.tile([P, f], _F32, tag="ajunk2")

                # DVE: products + fused free-axis sums (custom-DVE op:
                # out = (in0*1+0)*in1, accum_out = sum(out)).
                nc.vector.affine_mul_reduce(
                    out=nx_t[:], accum_out=stats_x[:, t : t + 1],
                    in0=x_t[:], in1=n_t[:], scale=1.0, bias=0.0,
                )
                nc.vector.affine_mul_reduce(
                    out=ny_t[:], accum_out=stats_y[:, t : t + 1],
                    in0=y_t[:], in1=n_t[:], scale=1.0, bias=0.0,
                )
                nc.vector.affine_mul_reduce(
                    out=junk_t[:], accum_out=stats_xy[:, t : t + 1],
                    in0=nx_t[:], in1=y_t[:], scale=1.0, bias=0.0,
                )

                # GPSIMD: nxx = nx * x.
                nc.gpsimd.tensor_tensor(
                    out=nxx_t[:], in0=nx_t[:], in1=x_t[:], op=_MULT
                )

                # ACT: free-axis sums of n and nxx via Copy + accumulate.
                nc.scalar.activation(
                    out=ajunk_t[:], in_=n_t[:], func=_COPY,
                    accum_out=stats_n[:, t : t + 1],
                )
                nc.scalar.activation(
                    out=ajunk2_t[:], in_=nxx_t[:], func=_COPY,
                    accum_out=stats_xx[:, t : t + 1],
                )

                # PE: fp32 diag-matmuls -> sum over chunks of ny_chunk^T @ y_chunk;
                # only the accumulated diagonal is meaningful (= sum n*y^2).
                for c in range(c128):
                    s = bass.ts(c, 128)
                    nc.tensor.matmul(
                        psum_yy[:],
                        ny_t[:, s],
                        y_t[:, s],
                        start=(rt == 0 and c == 0),
                        stop=(rt == n_iter - 1 and c == c128 - 1),
                    )

            nc.sync.dma_start(out=o_stats[0], in_=stats_x[:])
            nc.sync.dma_start(out=o_stats[1], in_=stats_y[:])
            nc.sync.dma_start(out=o_stats[2], in_=stats_xy[:])
            nc.sync.dma_start(out=o_stats[3], in_=stats_n[:])
            nc.sync.dma_start(out=o_stats[4], in_=stats_xx[:])
            # DMA cannot read PSUM: bounce through SBUF.
            sb_yy = accp.tile([P, P], _F32, tag="sbyy")
            nc.vector.tensor_copy(out=sb_yy[:], in_=psum_yy[:])
            nc.sync.dma_start(out=o_diag[:], in_=sb_yy[:])
            tick_t = accp.tile([P, 8], _F32, tag="tick")
            nc.sync.dma_start(out=tick_t[:], in_=tick[:])
            nc.sync.dma_start(out=tock[:], in_=tick_t[:])

    nc.finalize()
    return nc


_NC_CACHE = None


def _get_nc():
    global _NC_CACHE
    if _NC_CACHE is None:
        _NC_CACHE = build_nc()
    return _NC_CACHE


def combine_partials(results):
    """Host-side all-reduce of the per-core partials + correlation formula."""
    sum_x = sum_y = sum_prod = sum_n = sum_x2 = sum_y2 = 0.0
    for r in results:
        st = np.asarray(r["o_stats"], dtype=np.float64)
        sum_x += st[0].sum()
        sum_y += st[1].sum()
        sum_prod += st[2].sum()
        sum_n += st[3].sum()
        sum_x2 += st[4].sum()
        sum_y2 += np.diag(np.asarray(r["o_diag"], dtype=np.float64)).sum()
    numerator = sum_n * sum_prod - sum_x * sum_y
    denominator = np.sqrt(sum_n * sum_x2 - sum_x * sum_x) * np.sqrt(
        sum_n * sum_y2 - sum_y * sum_y
    )
    return np.asarray([numerator / denominator], dtype=np.float32)


def kernel(xs, ys, ns, **run_kwargs):
    xs = np.ascontiguousarray(np.asarray(xs, dtype=np.float32)).reshape(
        N_CORES, T, P, F
    )
    ys = np.ascontiguousarray(np.asarray(ys, dtype=np.float32)).reshape(
        N_CORES, T, P, F
    )
    ns = np.ascontiguousarray(np.asarray(ns, dtype=np.float32)).reshape(
        N_CORES, T, P, F
    )
    zt = np.zeros((P, 8), dtype=np.float32)
    in_maps = [
        {"xs": xs[c], "ys": ys[c], "ns": ns[c], "tick": zt} for c in range(N_CORES)
    ]
    res = run_bass_kernel_spmd(
        _get_nc(), in_maps, core_ids=list(range(N_CORES)), **run_kwargs
    )
    return combine_partials(res.results)
